# revision 9
# baseline (speedup 1.0000x reference)
"""Trainium2 Bass kernel for a 2-layer GAT (GATConv x2 + global mean pool + linear).

Strategy (8 NeuronCores, SPMD):
  - Nodes are dealt to cores by in-degree rank (rank r -> core r%8, slot r//8),
    so every core's slot s has ~equal degree => tight shared CSR capacity profile.
  - Edges live on the core that owns their DST. Per dst-group (128 slots) a
    padded CSR [128 dst x K columns] holds the in-edges; per-edge source rows
    are fetched with dma_gather (int16 idx => the 100352-row table is viewed in
    4 chunks of <=32768 rows; each group's columns are chunk-pure).
  - Per-node data (h | attention logits) is computed locally and AllGathered
    into a per-core DRAM table; pad slots point at dummy rows whose src-logit
    is -1e30 so exp(leakyrelu(...)) == 0 exactly.
  - Softmax runs per dst-partition row (max-subtraction is skipped: logits are
    O(1) so exp is safe in f32), aggregation is a broadcast-multiply + free-dim
    reduction on the Vector engine; layer outputs feed layer 2's table which is
    AllGathered again; mean-pool goes through a one-hot matmul on the Tensor
    engine and a final AllReduce.
"""
import sys, types, time

sys.path.insert(0, "/opt/trn_rl_repo")

import antenv  # noqa: E402
if not hasattr(antenv, "axon_hooks"):
    _m = types.ModuleType("antenv.axon_hooks")
    _m.get_axon_ntff_profile_hook = lambda: None
    sys.modules["antenv.axon_hooks"] = _m
    antenv.axon_hooks = _m

import numpy as np
import ml_dtypes
import concourse.bass as bass
import concourse.bacc as bacc
import concourse.tile as tile
import concourse.mybir as mybir
from concourse.masks import make_identity

# ---------------- problem constants (hardcoded; kernel.py must be self-contained)
N = 100000
IN = 64
HID = 16
HEADS = 4
G = 512
SLOPE = 0.2
NCORES = 8
SLOTS = 12544            # 98 groups x 128 (12500 real + 44 dummy slots per core)
NGRP = SLOTS // 128
NODESP = NCORES * SLOTS  # 100352 table rows
CHUNK_LIM = np.array([32768, 65536, 98304, NODESP])   # chunk of pid = searchsorted
VIEW_BASE = [0, 32768, 65536, 67584]                  # table view base per chunk
DUMMY_LOCAL = [12500, 4820, 9684, 32724]              # a dummy-slot row inside each view
COLS_PER_CALL = 7        # 896 tokens per dma_gather (57 descs; 2 in flight <= 128)
F32 = mybir.dt.float32
BF16 = mybir.dt.bfloat16
I16 = mybir.dt.int16


# ---------------------------------------------------------------- host side
def _host_prep(x, edge_index, batch):
    src = np.concatenate([edge_index[0], np.arange(N, dtype=np.int64)])
    dst = np.concatenate([edge_index[1], np.arange(N, dtype=np.int64)])
    deg = np.bincount(dst, minlength=N)
    order = np.argsort(-deg, kind="stable")
    rank = np.empty(N, np.int64)
    rank[order] = np.arange(N)
    core_of = rank % NCORES
    slot_of = rank // NCORES
    pid = core_of * SLOTS + slot_of

    spid = pid[src]
    schunk = np.searchsorted(CHUNK_LIM, spid, "right")
    slocal = spid - np.array(VIEW_BASE, np.int64)[schunk]
    dcore = core_of[dst]
    dslot = slot_of[dst]

    # per (core, slot, chunk) counts -> shared capacity profile K_gc
    cnt = np.zeros((NCORES, SLOTS, 4), np.int32)
    np.add.at(cnt, (dcore, dslot, schunk), 1)
    gcnt = cnt.reshape(NCORES, NGRP, 128, 4)
    Kgc = gcnt.max(axis=(0, 2))          # [NGRP, 4] shared profile
    Kg = Kgc.sum(axis=1)
    coloff = np.zeros((NGRP, 4), np.int64)
    coloff[:, 1:] = np.cumsum(Kgc, axis=1)[:, :-1]
    ntok = int(Kg.sum()) * 128

    # token values per core: [group columns x 128] int16, chunk-pure columns
    eorder = np.lexsort((schunk, dslot, dcore))
    es, ed, ec, el = (a[eorder] for a in (spid, dslot, dcore, slocal))
    ech = schunk[eorder]
    # j = rank within (core, slot, chunk)
    key = (ed * 4 + ech) + ec * (SLOTS * 4)
    uniq, first = np.unique(key, return_index=True)
    j = np.arange(len(key)) - np.repeat(first, np.diff(np.append(first, len(key))))

    idx16 = []
    gbase = np.concatenate([[0], np.cumsum(Kg)])   # column base per group
    for c in range(NCORES):
        m = ec == c
        gg = ed[m] // 128
        pp = ed[m] % 128
        col = gbase[gg] + coloff[gg, ech[m]] + j[m]
        val = np.empty(int(Kg.sum()) * 128, np.int16)
        # fill pads with per-column chunk dummy
        colchunk = np.empty(int(Kg.sum()), np.int8)
        for g in range(NGRP):
            for ch in range(4):
                colchunk[gbase[g] + coloff[g, ch]: gbase[g] + coloff[g, ch] + Kgc[g, ch]] = ch
        dl = np.array(DUMMY_LOCAL, np.int16)[colchunk]
        val = np.repeat(dl, 128).astype(np.int16)
        val[col * 128 + pp] = el[m].astype(np.int16)
        # wrap: token t -> idx[t%16, t//16]
        blk = val.reshape(-1, 16).T.astype(np.int16)   # [16, ntok/16]
        idx16.append(np.ascontiguousarray(np.tile(blk, (8, 1))))

    # xT per core [IN, SLOTS]
    xT = np.zeros((NCORES, IN, SLOTS), np.float32)
    for c in range(NCORES):
        nodes = np.where(core_of == c)[0]
        xT[c][:, slot_of[nodes]] = x[nodes].T

    # pooling one-hot [NGRP, 128, G] bf16 + inv counts
    poolind = np.zeros((NCORES, SLOTS, G), np.float32)
    for c in range(NCORES):
        nodes = np.where(core_of == c)[0]
        poolind[c, slot_of[nodes], batch[nodes]] = 1.0
    poolind = poolind.reshape(NCORES, NGRP, 128, G).astype(ml_dtypes.bfloat16)
    counts = np.bincount(batch, minlength=G).astype(np.float32)
    invcnt = np.tile((1.0 / np.maximum(counts, 1.0))[None, :], (HID, 1)).astype(np.float32)

    meta = dict(Kgc=Kgc, Kg=Kg, coloff=coloff, gbase=gbase, ntok=ntok)
    return meta, idx16, xT, poolind, invcnt


# ---------------------------------------------------------------- device build
def _build(meta):
    Kgc, Kg, gbase = meta["Kgc"], meta["Kg"], meta["gbase"]
    NIDX16 = int(Kg.sum()) * 8          # idx columns (int16)
    W1E = 72                            # h(64) | als(4) | ald(4)
    W2E = 18                            # h2(16) | als2 | ald2
    TB1W = 128                          # table1 row elems (bf16, 256B)
    TB2W = 64                           # table2 row elems (f32, 256B)

    nc = bacc.Bacc(None, target_bir_lowering=False)
    xT = nc.declare_dram_parameter("xT", [IN, SLOTS], F32, isOutput=False)
    wext = nc.declare_dram_parameter("wext", [IN, W1E], F32, isOutput=False)
    w2ext = nc.declare_dram_parameter("w2ext", [IN, W2E], F32, isOutput=False)
    b1t = nc.declare_dram_parameter("b1t", [128, 64], F32, isOutput=False)
    b2t = nc.declare_dram_parameter("b2t", [128, HID], F32, isOutput=False)
    idx = nc.declare_dram_parameter("idx", [128, NIDX16], I16, isOutput=False)
    poolind = nc.declare_dram_parameter("poolind", [NGRP, 128, G], BF16, isOutput=False)
    invcnt = nc.declare_dram_parameter("invcnt", [HID, G], F32, isOutput=False)
    wc = nc.declare_dram_parameter("wc", [HID, 1], F32, isOutput=False)
    bc = nc.declare_dram_parameter("bc", [1, 1], F32, isOutput=False)
    neg = nc.declare_dram_parameter("neg", [44, 4], BF16, isOutput=False)
    negf = nc.declare_dram_parameter("negf", [44, 1], F32, isOutput=False)
    out = nc.declare_dram_parameter("out", [1, G], F32, isOutput=True)

    table1 = nc.dram_tensor("table1", [NODESP, TB1W], BF16, addr_space="Shared")
    table2 = nc.dram_tensor("table2", [NODESP, TB2W], F32, addr_space="Shared")
    ar_out = nc.dram_tensor("ar_out", [HID, G], F32, addr_space="Shared")

    cc_ins = {}

    with tile.TileContext(nc) as tc:
        with (
            tc.tile_pool(name="persist", bufs=1) as pp,
            tc.tile_pool(name="work", bufs=2) as wp,
            tc.tile_pool(name="psum", bufs=2, space="PSUM") as psp,
            tc.tile_pool(name="psum1", bufs=1, space="PSUM") as psp1,
            tc.tile_pool(name="dram", bufs=1, space="DRAM") as dp,
        ):
            # --- persistent loads
            wext_sb = pp.tile([IN, W1E], F32)
            nc.sync.dma_start(wext_sb[:], wext[:, :])
            w2ext_sb = pp.tile([IN, W2E], F32)
            nc.sync.dma_start(w2ext_sb[:], w2ext[:, :])
            b1_sb = pp.tile([128, 64], F32)
            nc.sync.dma_start(b1_sb[:], b1t[:, :])
            b2_sb = pp.tile([128, HID], F32)
            nc.sync.dma_start(b2_sb[:], b2t[:, :])
            wc_sb = pp.tile([HID, 1], F32)
            nc.sync.dma_start(wc_sb[:], wc[:, :])
            bc_sb = pp.tile([1, 1], F32)
            nc.sync.dma_start(bc_sb[:], bc[:, :])
            invc_sb = pp.tile([HID, G], F32)
            nc.sync.dma_start(invc_sb[:], invcnt[:, :])
            ident = pp.tile([128, 128], F32)
            make_identity(nc, ident[:])

            ald_sb = pp.tile([128, NGRP, HEADS], F32)
            z1_all = pp.tile([128, NGRP, 64], F32)
            ald2_sb = pp.tile([128, NGRP], F32)
            z2_all = pp.tile([128, NGRP, HID], F32)
            t1in = dp.tile([SLOTS, TB1W], BF16)
            t2in = dp.tile([SLOTS, TB2W], F32)
            ar_in = dp.tile([HID, G], F32)

            # ---------------- phase A: hext = x @ Wext per own slot
            for g in range(NGRP):
                xg = wp.tile([IN, 128], F32, tag="xg")
                nc.sync.dma_start(xg[:], xT[:, g * 128:(g + 1) * 128])
                ps = psp.tile([128, W1E], F32, tag="psA")
                nc.tensor.matmul(ps[:], xg[:], wext_sb[:], start=True, stop=True)
                st1 = wp.tile([128, 68], BF16, tag="st1")
                nc.vector.tensor_copy(st1[:], ps[:, 0:68])
                nc.vector.tensor_copy(ald_sb[:, g, :], ps[:, 68:72])
                nc.sync.dma_start(t1in[g * 128:(g + 1) * 128, 0:68], st1[:])
            nc.sync.dma_start(t1in[12500:12544, 64:68], neg[:, :])
            cc1 = nc.gpsimd.collective_compute(
                "AllGather", mybir.AluOpType.bypass,
                replica_groups=[list(range(NCORES))],
                ins=[t1in[:].opt()], outs=[table1[:, :].opt()])
            cc_ins["ag1"] = cc1.ins if hasattr(cc1, "ins") else cc1

            # ---------------- phase B: layer-1 edge aggregation
            def edge_layer(table, TBW, nfeat, nheads, ald_t, bias_sb, zout, layer):
                alcol = nfeat  # src-logit column(s) start
                for g in range(NGRP):
                    K = int(Kg[g])
                    idx_t = wp.tile([128, 8 * int(Kg.max())], I16, tag="idx")
                    nc.sync.dma_start(idx_t[:, 0:8 * K],
                                      idx[:, 8 * gbase[g]: 8 * (gbase[g] + K)])
                    gt = wp.tile([128, int(Kg.max()) * TBW], table.dtype, tag="gt")
                    gtv = gt[:, 0:K * TBW].rearrange("p (k e) -> p k e", e=TBW)
                    for c in range(4):
                        kc = int(Kgc[g, c])
                        off = int(meta_coloff[g, c])
                        base = VIEW_BASE[c]
                        a = 0
                        while a < kc:
                            b = min(a + COLS_PER_CALL, kc)
                            T = (b - a) * 128
                            ins_g = nc.gpsimd.dma_gather(
                                gtv[:, off + a: off + b, :],
                                table[base: base + 32768, :],
                                idx_t[:, 8 * (off + a): 8 * (off + b)],
                                T, T, TBW)
                            cc_ins.setdefault(f"gathers{layer}", []).append(ins_g)
                            a = b

                    # e = lrelu(als + ald); w = exp(e)  [128, nheads, K]
                    e_t = wp.tile([128, nheads * int(Kg.max())], F32, tag="e")
                    ev = e_t[:, 0:nheads * K].rearrange("p (h k) -> p h k", k=K)
                    for h in range(nheads):
                        nc.vector.tensor_scalar_add(
                            ev[:, h, :],
                            gtv[:, :, alcol + h],
                            ald_t[:, g, h:h + 1] if nheads > 1 else ald_t[:, g:g + 1])
                    es_t = wp.tile([128, nheads * int(Kg.max())], F32, tag="es")
                    nc.vector.tensor_scalar_mul(es_t[:, 0:nheads * K],
                                                e_t[:, 0:nheads * K], SLOPE)
                    nc.vector.tensor_tensor(out=e_t[:, 0:nheads * K],
                                            in0=e_t[:, 0:nheads * K],
                                            in1=es_t[:, 0:nheads * K],
                                            op=mybir.AluOpType.max)
                    wdt = BF16 if table.dtype == BF16 else F32
                    w_t = wp.tile([128, nheads * int(Kg.max())], wdt, tag="w")
                    wv = w_t[:, 0:nheads * K].rearrange("p (h k) -> p h k", k=K)
                    nc.scalar.activation(w_t[:, 0:nheads * K], e_t[:, 0:nheads * K],
                                         mybir.ActivationFunctionType.Exp)
                    den = wp.tile([128, nheads], F32, tag="den")
                    nc.vector.reduce_sum(den[:], wv[:, :, :], axis=mybir.AxisListType.X)
                    dinv = wp.tile([128, nheads], F32, tag="dinv")
                    nc.vector.tensor_scalar_add(den[:], den[:], 1e-16)
                    nc.vector.reciprocal(dinv[:], den[:])

                    # m = w (bcast 16) * h ; msum = sum_j m
                    CH = nfeat // nheads
                    m_t = wp.tile([128, int(Kg.max()) * nfeat],
                                  BF16 if table.dtype == BF16 else F32, tag="m")
                    mv = m_t[:, 0:K * nfeat].rearrange("p (k h c) -> p k h c",
                                                       h=nheads, c=CH)
                    for h in range(nheads):
                        nc.vector.tensor_tensor(
                            out=mv[:, :, h, :],
                            in0=gtv[:, :, h * CH:(h + 1) * CH],
                            in1=wv[:, h:h + 1, :].rearrange("p o k -> p k o").to_broadcast(
                                [128, K, CH]),
                            op=mybir.AluOpType.mult)
                    msum = wp.tile([128, nfeat], F32, tag="ms")
                    nc.vector.reduce_sum(
                        msum[:].rearrange("p (h c) -> p h c", c=CH),
                        mv.rearrange("p k h c -> p h c k"),
                        axis=mybir.AxisListType.X)
                    # out = msum * dinv (+bias), elu
                    y = wp.tile([128, nfeat], F32, tag="y")
                    nc.vector.tensor_tensor(
                        out=y[:].rearrange("p (h c) -> p h c", c=CH),
                        in0=msum[:].rearrange("p (h c) -> p h c", c=CH),
                        in1=dinv[:, :, None].to_broadcast([128, nheads, CH])
                        if False else dinv[:].rearrange("p (h o) -> p h o", o=1).to_broadcast(
                            [128, nheads, CH]),
                        op=mybir.AluOpType.mult)
                    nc.vector.tensor_add(y[:], y[:], bias_sb[:])
                    tneg = wp.tile([128, nfeat], F32, tag="tn")
                    nc.vector.tensor_scalar_min(tneg[:], y[:], 0.0)
                    ex = wp.tile([128, nfeat], F32, tag="ex")
                    nc.scalar.activation(ex[:], tneg[:], mybir.ActivationFunctionType.Exp)
                    nc.vector.tensor_scalar_add(ex[:], ex[:], -1.0)
                    nc.vector.tensor_scalar_max(y[:], y[:], 0.0)
                    nc.vector.tensor_add(zout[:, g, :], y[:], ex[:])

            global meta_coloff
            meta_coloff = meta["coloff"]
            edge_layer(table1, 128, 64, HEADS, ald_sb, b1_sb, z1_all, 1)

            # ---------------- phase C: layer-2 node compute (z1 @ W2ext)
            for g in range(NGRP):
                pst = psp.tile([64, 128], F32, tag="psT")
                nc.tensor.transpose(pst[:], z1_all[:, g, :], ident[:])
                z1T = wp.tile([64, 128], F32, tag="z1T")
                nc.vector.tensor_copy(z1T[:], pst[:])
                ps2 = psp.tile([128, W2E], F32, tag="ps2")
                nc.tensor.matmul(ps2[:], z1T[:], w2ext_sb[:], start=True, stop=True)
                st2 = wp.tile([128, 17], F32, tag="st2")
                nc.vector.tensor_copy(st2[:], ps2[:, 0:17])
                nc.vector.tensor_copy(ald2_sb[:, g:g + 1], ps2[:, 17:18])
                nc.sync.dma_start(t2in[g * 128:(g + 1) * 128, 0:17], st2[:])
            if g == NGRP - 1:
                pass
            nc.sync.dma_start(t2in[12500:12544, 16:17], negf[:, :])
            cc2 = nc.gpsimd.collective_compute(
                "AllGather", mybir.AluOpType.bypass,
                replica_groups=[list(range(NCORES))],
                ins=[t2in[:].opt()], outs=[table2[:, :].opt()])
            cc_ins["ag2"] = cc2.ins if hasattr(cc2, "ins") else cc2

            # ---------------- phase D: layer-2 edge aggregation
            edge_layer(table2, TB2W, HID, 1, ald2_sb, b2_sb, z2_all, 2)

            # ---------------- phase E: pooling + final linear
            pool_ps = psp1.tile([HID, G], F32)
            for g in range(NGRP):
                z2b = wp.tile([128, HID], BF16, tag="z2b")
                nc.vector.tensor_copy(z2b[:], z2_all[:, g, :])
                pind = wp.tile([128, G], BF16, tag="pind")
                nc.sync.dma_start(pind[:], poolind[g, :, :])
                nc.tensor.matmul(pool_ps[:], z2b[:], pind[:],
                                 start=(g == 0), stop=(g == NGRP - 1))
            pool_sb = pp.tile([HID, G], F32)
            nc.vector.tensor_copy(pool_sb[:], pool_ps[:])
            nc.sync.dma_start(ar_in[:], pool_sb[:])
            cc3 = nc.gpsimd.collective_compute(
                "AllReduce", mybir.AluOpType.add,
                replica_groups=[list(range(NCORES))],
                ins=[ar_in[:].opt()], outs=[ar_out[:, :].opt()])
            cc_ins["ar"] = cc3.ins if hasattr(cc3, "ins") else cc3

            pooled = pp.tile([HID, G], F32)
            dma_pool = nc.sync.dma_start(pooled[:], ar_out[:, :])
            nc.vector.tensor_mul(pooled[:], pooled[:], invc_sb[:])
            ps_out = psp1.tile([1, G], F32)
            nc.tensor.matmul(ps_out[:], wc_sb[:], pooled[:], start=True, stop=True)
            res = pp.tile([1, G], F32)
            nc.vector.tensor_tensor(out=res[:], in0=ps_out[:],
                                    in1=bc_sb[:].to_broadcast([1, G]),
                                    op=mybir.AluOpType.add)
            nc.sync.dma_start(out[:, :], res[:])

            # explicit ordering: consumers of Shared collective outputs
            from concourse.bass import _add_dep_helper
            for gg in cc_ins["gathers1"]:
                _add_dep_helper(gg.ins, cc_ins["ag1"], sync=True)
            for gg in cc_ins["gathers2"]:
                _add_dep_helper(gg.ins, cc_ins["ag2"], sync=True)
            _add_dep_helper(dma_pool.ins, cc_ins["ar"], sync=True)

    nc.finalize()
    return nc


# ---------------------------------------------------------------- runner
def _make_spmd_fn(nc, n_cores=8):
    import jax
    from concourse.bass2jax import (_bass_exec_p, install_neuronx_cc_hook,
                                    partition_id_tensor)
    from jax.sharding import Mesh, PartitionSpec, NamedSharding
    from jax.experimental.shard_map import shard_map

    install_neuronx_cc_hook()
    partition_name = nc.partition_id_tensor.name if nc.partition_id_tensor else None
    in_names, out_names, out_avals, zero_outs = [], [], [], []
    for alloc in nc.m.functions[0].allocations:
        if not isinstance(alloc, mybir.MemoryLocationSet):
            continue
        name = alloc.memorylocations[0].name
        if alloc.kind == "ExternalInput":
            if name != partition_name:
                in_names.append(name)
        elif alloc.kind == "ExternalOutput":
            out_names.append(name)
            shape = tuple(alloc.tensor_shape)
            dtype = mybir.dt.np(alloc.dtype)
            out_avals.append(jax.core.ShapedArray(shape, dtype))
            zero_outs.append(np.zeros(shape, dtype))
    n_params = len(in_names)
    all_in = list(in_names) + list(out_names)
    if partition_name is not None:
        all_in.append(partition_name)

    def _body(*args):
        operands = list(args)
        if partition_name is not None:
            operands.append(partition_id_tensor())
        return tuple(_bass_exec_p.bind(
            *operands, out_avals=tuple(out_avals), in_names=tuple(all_in),
            out_names=tuple(out_names), lowering_input_output_aliases=(),
            sim_require_finite=False, sim_require_nnan=False, nc=nc))

    devices = jax.devices()[:n_cores]
    mesh = Mesh(np.asarray(devices), ("core",))
    specs = (PartitionSpec("core"),)
    sharded = jax.jit(
        shard_map(_body, mesh=mesh, in_specs=specs * (n_params + len(out_names)),
                  out_specs=specs * len(out_names), check_rep=False),
        keep_unused=True)

    def fn(in_maps):
        concat = [np.concatenate([np.asarray(in_maps[c][nm]) for c in range(n_cores)],
                                 axis=0) for nm in in_names]
        zeros = [np.zeros((n_cores * z.shape[0], *z.shape[1:]), z.dtype)
                 for z in zero_outs]
        outs = sharded(*concat, *zeros)
        jax.block_until_ready(outs)
        return [{nm: np.asarray(outs[i]).reshape(n_cores, *out_avals[i].shape)[c]
                 for i, nm in enumerate(out_names)} for c in range(n_cores)]

    return fn


_CACHE = {}


def kernel(**inputs):
    x = np.asarray(inputs["x"], np.float32)
    edge_index = np.asarray(inputs["edge_index"], np.int64)
    batch = np.asarray(inputs["batch"], np.int64)
    W1 = np.asarray(inputs["W1"], np.float32)
    a1_src = np.asarray(inputs["a1_src"], np.float32)
    a1_dst = np.asarray(inputs["a1_dst"], np.float32)
    b1 = np.asarray(inputs["b1"], np.float32)
    W2 = np.asarray(inputs["W2"], np.float32)
    a2_src = np.asarray(inputs["a2_src"], np.float32)
    a2_dst = np.asarray(inputs["a2_dst"], np.float32)
    b2 = np.asarray(inputs["b2"], np.float32)
    Wc = np.asarray(inputs["Wc"], np.float32)
    bc = np.asarray(inputs["bc"], np.float32)

    meta, idx16, xT, poolind, invcnt = _host_prep(x, edge_index, batch)

    # fold attention vectors into the node matmul (weight packing only)
    wext = np.zeros((IN, 72), np.float32)
    wext[:, 0:64] = W1
    for h in range(HEADS):
        wext[:, 64 + h] = W1[:, h * HID:(h + 1) * HID] @ a1_src[h]
        wext[:, 68 + h] = W1[:, h * HID:(h + 1) * HID] @ a1_dst[h]
    w2ext = np.zeros((IN, 18), np.float32)
    w2ext[:, 0:16] = W2
    w2ext[:, 16] = W2 @ a2_src[0]
    w2ext[:, 17] = W2 @ a2_dst[0]
    b1t = np.tile(b1[None, :], (128, 1)).astype(np.float32)
    b2t = np.tile(b2[None, :], (128, 1)).astype(np.float32)

    nc = _build(meta)
    fn = _make_spmd_fn(nc)
    in_maps = []
    for c in range(NCORES):
        in_maps.append(dict(
            xT=xT[c], wext=wext, w2ext=w2ext, b1t=b1t, b2t=b2t,
            idx=idx16[c], poolind=poolind[c], invcnt=invcnt,
            wc=Wc.astype(np.float32).reshape(HID, 1),
            bc=bc.astype(np.float32).reshape(1, 1),
            neg=np.full((44, 4), -1e30, ml_dtypes.bfloat16),
            negf=np.full((44, 1), -1e30, np.float32)))
    res = fn(in_maps)
    out = res[0]["out"].reshape(G, 1).astype(np.float32)
    kernel._last_fn = fn
    kernel._last_in_maps = in_maps
    return out
